# revision 2
# baseline (speedup 1.0000x reference)
"""ChildSum TreeLSTM + attention — quad-local SPMD Trainium2 kernel.

v2: all communication stays inside a 4-core quad (cross-die remote DMA
on this platform costs ~8x a same-die send and is flaky). Cores 0-3 and
4-7 run identical replicated quads; output is read from core 0.

Per-quad sharding (j = core % 4):
  - mem 1024 -> 4 slices of 256 (own chunks 2j, 2j+1 in [128, 8] col form)
  - attention rows 512 -> 128 rows per core
  - per step: phase A gates (64 MMs) -> gates/c/h -> w-gemv (16 MMs)
    -> SYNC1 [h one-hot | w partial] [128,16] -> tanh (8 ACT) + scores
    (8 MMs) -> exp -> numer (8 MMs) -> SYNC2 [numer | e] [128,9]
    -> h_att = csum + h_new_full - numer/Z
  - SWDGE preps are hoisted before the payload writes (descriptors are
    address-only; data is read at trigger time), so only trigger+flight
    is on the critical path.
  - big stationaries (wrec/w1t/hrows/tT/wa) in bf16: PE time here is
    dominated by LD_WEIGHTS, ~1.5x cheaper in bf16.
"""
import numpy as np
from contextlib import ExitStack

import ml_dtypes

import concourse.bass as bass
import concourse.tile as tile
from concourse import bacc, mybir
from concourse.bass import create_sync_update
from concourse.tile_rust import add_dep_helper

F32 = mybir.dt.float32
BF16 = mybir.dt.bfloat16
AF = mybir.ActivationFunctionType
N_CORES = 8
QUAD = 4
MEM = 1024
IN_DIM = 1024
MROWS = 512
KC = MEM // 128          # 8 col chunks
OC = 2                   # own mem chunks per core (256 dims)
RPC = MROWS // QUAD      # 128 attention rows per core

_EXTERNAL_SEMS: list = []
_OrigCoreSim = tile.CoreSim


class _SchedCoreSim(_OrigCoreSim):
    def __init__(self, *a, **kw):
        super().__init__(*a, **kw)
        for sem in _EXTERNAL_SEMS:
            self.update_semaphore(create_sync_update(sem, 1 << 22))


tile.CoreSim = _SchedCoreSim


def _prep4(nc, out_slot_of, in_ap, remote_sem, local_sem, chain):
    """4 single-dest intra-quad broadcasts: slot k -> peer (own_tpb ^ k).

    Descriptors only; the caller fires them later with trigger_dma.
    Preps of consecutive syncs must hit the SWDGE ring in trace order.
    """
    prev = chain[0]
    for k in range(QUAD):
        rdests = [None] * 8
        rdests[k] = (0, k)
        inst = nc.gpsimd.remote_dma_broadcast(
            out_ap=out_slot_of(k),
            in_ap=in_ap,
            remote_sem=remote_sem,
            local_sem=local_sem,
            rdests=rdests,
        )
        if prev is not None:
            add_dep_helper(inst.ins, prev, False, "swdge ring order")
        prev = inst.ins
    chain[0] = prev
    return prev


def _trigger(nc, chain, payload_insts):
    trig = nc.gpsimd.trigger_dma(count=None)
    add_dep_helper(trig.ins, chain[0], False, "swdge ring order")
    for pi in payload_insts:
        # cross-engine (DVE/ACT -> Pool): needs a real semaphore edge
        add_dep_helper(trig.ins, pi, True, "payload ready")
    chain[0] = trig.ins
    return trig


def build_nc(T: int, wdt=BF16, t_run: int | None = None, no_comm: bool = False):
    del _EXTERNAL_SEMS[:]
    nc = bacc.Bacc()

    dp = lambda n, s, dt=F32: nc.declare_dram_parameter(n, s, dt, isOutput=False)
    xT = dp("xT", [128, KC * T])
    wx = dp("wx", [128, 4 * KC * OC * 128])
    wrec = dp("wrec", [128, 4 * KC * OC * 128], wdt)
    w1t = dp("w1t", [128, KC * OC * 128], wdt)
    w2t = dp("w2t", [128, KC * KC * 128])
    hTs = dp("hT", [128, KC * MROWS])
    hTown = dp("hTown", [128, KC * RPC])
    hrows = dp("hrows", [RPC, MEM], wdt)
    wa = dp("wa", [128, KC], wdt)
    bias_x = dp("bias_x", [128, 4 * OC])
    bias2 = dp("bias2", [128, KC])
    mask = dp("mask", [128, 16])  # cols 0:8 one-hot 2j, cols 8:16 one-hot 2j+1
    hout = nc.declare_dram_parameter("hout", [T, 128, KC], F32, isOutput=True)

    with tile.TileContext(nc) as tc, ExitStack() as ctx:
        sem1 = ctx.enter_context(nc.semaphore("rdma_sem1"))
        sem2 = ctx.enter_context(nc.semaphore("rdma_sem2"))
        lsem1 = ctx.enter_context(nc.semaphore("rdma_lsem1"))
        lsem2 = ctx.enter_context(nc.semaphore("rdma_lsem2"))
        _EXTERNAL_SEMS.extend([sem1, sem2, lsem1, lsem2])

        comm = ctx.enter_context(tc.tile_pool(name="comm", bufs=1))
        pay1 = [comm.tile([128, 16], F32, name=f"pay1_{p}", tag=f"pay1_{p}") for p in range(2)]
        rec1 = [comm.tile([128, 16 * QUAD], F32, name=f"rec1_{p}", tag=f"rec1_{p}") for p in range(2)]
        pay2 = [comm.tile([128, 9], F32, name=f"pay2_{p}", tag=f"pay2_{p}") for p in range(2)]
        rec2 = [comm.tile([128, 9 * QUAD], F32, name=f"rec2_{p}", tag=f"rec2_{p}") for p in range(2)]

        const = ctx.enter_context(tc.tile_pool(name="const", bufs=1))
        wrec_sb = const.tile([128, 4 * KC * OC * 128], wdt, tag="wrec")
        w1t_sb = const.tile([128, KC * OC * 128], wdt, tag="w1t")
        hrows_sb = const.tile([RPC, MEM], wdt, tag="hrows")
        wa_sb = const.tile([128, KC], wdt, tag="wa")
        hw2T_sb = const.tile([128, KC * RPC], F32, tag="hw2T")
        xproj_sb = const.tile([128, 4 * OC * T], F32, tag="xproj")
        csum_sb = const.tile([128, KC], F32, tag="csum")
        ones_sb = const.tile([128, 128], F32, tag="ones")
        mask_sb = const.tile([128, 16], F32, tag="mask")

        nc.sync.dma_start(wrec_sb[:, :], wrec.ap())
        nc.sync.dma_start(w1t_sb[:, :], w1t.ap())
        nc.sync.dma_start(hrows_sb[:, :], hrows.ap())
        nc.sync.dma_start(wa_sb[:, :], wa.ap())
        nc.sync.dma_start(mask_sb[:, :], mask.ap())
        nc.vector.memset(ones_sb[:, :], 1.0)
        for p in range(2):
            nc.vector.memset(pay1[p][:, :], 0.0)
            nc.vector.memset(pay2[p][:, :], 0.0)
        if no_comm:
            for p in range(2):
                nc.vector.memset(rec1[p][:, :], 1.0)
                nc.vector.memset(rec2[p][:, :], 1.0)

        # ---------- device precompute ----------
        with tc.tile_pool(name="pre", bufs=1) as pre, \
             tc.tile_pool(name="prepsum", bufs=1, space="PSUM") as pps:
            xT_sb = pre.tile([128, KC * T], F32, tag="xT")
            wx_sb = pre.tile([128, 4 * KC * OC * 128], F32, tag="wx")
            w2t_sb = pre.tile([128, KC * KC * 128], F32, tag="w2t")
            hT_sb = pre.tile([128, KC * MROWS], F32, tag="hT")
            hTown_sb = pre.tile([128, KC * RPC], F32, tag="hTown")
            bx_sb = pre.tile([128, 4 * OC], F32, tag="bias_x")
            b2_sb = pre.tile([128, KC], F32, tag="bias2")
            nc.sync.dma_start(xT_sb[:, :], xT.ap())
            nc.sync.dma_start(wx_sb[:, :], wx.ap())
            nc.sync.dma_start(w2t_sb[:, :], w2t.ap())
            nc.sync.dma_start(hT_sb[:, :], hTs.ap())
            nc.sync.dma_start(hTown_sb[:, :], hTown.ap())
            nc.sync.dma_start(bx_sb[:, :], bias_x.ap())
            nc.sync.dma_start(b2_sb[:, :], bias2.ap())

            # xproj[(g,o)]: [128, T] = sum_k Wx tile (g,k,o)^T @ xT[k]
            for g in range(4):
                for o in range(OC):
                    ps = pps.tile([128, T], F32, tag="ps_x")
                    for k in range(KC):
                        idx = (g * KC + k) * OC + o
                        nc.tensor.matmul(
                            ps[:, :],
                            wx_sb[:, idx * 128:(idx + 1) * 128],
                            xT_sb[:, k * T:(k + 1) * T],
                            start=(k == 0), stop=(k == KC - 1),
                        )
                        nc.vector.tensor_scalar_add(
                            xproj_sb[:, (g * OC + o) * T:(g * OC + o + 1) * T],
                            ps[:, :], bx_sb[:, g * OC + o:g * OC + o + 1],
                        ) if k == KC - 1 else None

            # hw2T c-chunk: [128, RPC] = sum_k W2[c,k]^T @ hTown[k]
            for c in range(KC):
                ps2 = pps.tile([128, RPC], F32, tag="ps_h")
                for k in range(KC):
                    nc.tensor.matmul(
                        ps2[:, :],
                        w2t_sb[:, (c * KC + k) * 128:(c * KC + k + 1) * 128],
                        hTown_sb[:, k * RPC:(k + 1) * RPC],
                        start=(k == 0), stop=(k == KC - 1),
                    )
                nc.vector.tensor_scalar_add(
                    hw2T_sb[:, c * RPC:(c + 1) * RPC], ps2[:, :], b2_sb[:, c:c + 1]
                )

            for m in range(KC):
                nc.vector.reduce_sum(
                    csum_sb[:, m:m + 1],
                    hT_sb[:, m * MROWS:(m + 1) * MROWS],
                    axis=mybir.AxisListType.X,
                )

        # ---------- state & per-step pools ----------
        sp = ctx.enter_context(tc.tile_pool(name="step", bufs=2))
        psp = ctx.enter_context(tc.tile_pool(name="spsum", bufs=1, space="PSUM"))

        chain = [None]
        hcol = sp.tile([128, KC], F32, tag="hcol")
        ccol = sp.tile([128, OC], F32, tag="ccol")
        nc.vector.memset(hcol[:, :], 0.0)
        nc.vector.memset(ccol[:, :], 0.0)

        for t in range(t_run if t_run is not None else T):
            par = t & 1
            # hoisted SYNC1 preps (descriptors only; data read at trigger)
            if not no_comm:
                _prep4(nc, lambda k: rec1[par][:, k * 16:(k + 1) * 16],
                       pay1[par][:, 0:16], sem1, lsem1, chain)

            # ---- phase A: gate pre-activations for own 2 chunks ----
            if wdt != F32:
                hcol_w = sp.tile([128, KC], wdt, tag="hcol_w")
                nc.vector.tensor_copy(hcol_w[:, :], hcol[:, :])
            else:
                hcol_w = hcol
            psA = psp.tile([128, 4 * OC], F32, tag="psA")
            for g in range(4):
                for o in range(OC):
                    for k in range(KC):
                        idx = (g * KC + k) * OC + o
                        nc.tensor.matmul(
                            psA[:, g * OC + o:g * OC + o + 1],
                            wrec_sb[:, idx * 128:(idx + 1) * 128],
                            hcol_w[:, k:k + 1],
                            start=(k == 0), stop=(k == KC - 1),
                        )
            # ---- gates ----
            gates = sp.tile([128, 4 * OC], F32, tag="gates")
            for g, fn in ((0, AF.Sigmoid), (1, AF.Sigmoid), (2, AF.Tanh), (3, AF.Sigmoid)):
                for o in range(OC):
                    col = g * OC + o
                    nc.scalar.activation(
                        gates[:, col:col + 1], psA[:, col:col + 1], fn,
                        bias=xproj_sb[:, col * T + t:col * T + t + 1],
                    )
            iu = sp.tile([128, OC], F32, tag="iu")
            nc.vector.tensor_mul(iu[:, :], gates[:, 0:2], gates[:, 4:6])
            ccol_new = sp.tile([128, OC], F32, tag="ccol")
            nc.vector.tensor_mul(ccol_new[:, :], gates[:, 6:8], ccol[:, :])
            nc.vector.tensor_add(ccol_new[:, :], ccol_new[:, :], iu[:, :])
            ccol = ccol_new
            tanh_c = sp.tile([128, OC], F32, tag="tanh_c")
            nc.scalar.activation(tanh_c[:, :], ccol[:, :], AF.Tanh)
            h_new = sp.tile([128, OC], F32, tag="h_new")
            hn_inst = nc.vector.tensor_mul(h_new[:, :], gates[:, 2:4], tanh_c[:, :])

            # ---- w-gemv: psW[:, m] = sum_o W1[own o]^T @ h_new[:, o] ----
            psW = psp.tile([128, KC], F32, tag="psW")
            if wdt != F32:
                h_new_w = sp.tile([128, OC], wdt, tag="h_new_w")
                nc.vector.tensor_copy(h_new_w[:, :], h_new[:, :])
            else:
                h_new_w = h_new
            for m in range(KC):
                for o in range(OC):
                    idx = m * OC + o
                    nc.tensor.matmul(
                        psW[:, m:m + 1],
                        w1t_sb[:, idx * 128:(idx + 1) * 128],
                        h_new_w[:, o:o + 1],
                        start=(o == 0), stop=(o == OC - 1),
                    )
            if t >= 2 and not no_comm:
                lw1 = nc.vector.wait_ge(lsem1, 64 * (t - 1))
                add_dep_helper(lw1.ins, hn_inst.ins, False, "anchor lsem1 wait")

            tmpE = sp.tile([128, KC], F32, tag="tmpE")
            tmpO = sp.tile([128, KC], F32, tag="tmpO")
            mE = nc.vector.tensor_scalar_mul(tmpE[:, :], mask_sb[:, 0:8], h_new[:, 0:1])
            mO = nc.vector.tensor_scalar_mul(tmpO[:, :], mask_sb[:, 8:16], h_new[:, 1:2])
            cph = nc.vector.tensor_add(pay1[par][:, 0:8], tmpE[:, :], tmpO[:, :])
            cp1 = nc.vector.tensor_copy(pay1[par][:, 8:16], psW[:, :])
            if t >= 2 and not no_comm:
                add_dep_helper(cph.ins, lw1.ins, False, "pay1 WAR")
                add_dep_helper(cp1.ins, lw1.ins, False, "pay1 WAR")

            # ---- fire SYNC1 ----
            if not no_comm:
                _trigger(nc, chain, [cph.ins, cp1.ins])
                w1_inst = nc.vector.wait_ge(sem1, 8 * (t + 1))
                add_dep_helper(w1_inst.ins, cp1.ins, False, "anchor sem1 wait")

            hnew_full = sp.tile([128, KC], F32, tag="hnew_full")
            r1 = rec1[par][:, :].rearrange("p (s c) -> p c s", s=QUAD)
            i1 = nc.vector.reduce_sum(hnew_full[:, :], r1[:, 0:8, :], axis=mybir.AxisListType.X)
            w_sum = sp.tile([128, KC], F32, tag="w_sum")
            i2 = nc.vector.reduce_sum(w_sum[:, :], r1[:, 8:16, :], axis=mybir.AxisListType.X)
            if not no_comm:
                add_dep_helper(i1.ins, w1_inst.ins, False, "gate recv1")
                add_dep_helper(i2.ins, w1_inst.ins, False, "gate recv1")
                # hoisted SYNC2 preps (after trig1 in ring order)
                _prep4(nc, lambda k: rec2[par][:, k * 9:(k + 1) * 9],
                       pay2[par][:, 0:9], sem2, lsem2, chain)

            # ---- attention rows (own 128): tT = tanh(hw2T + w) ----
            tT = sp.tile([128, KC * RPC], wdt, tag="tT")
            tanh_insts = []
            for c in range(KC):
                tanh_insts.append(nc.scalar.activation(
                    tT[:, c * RPC:(c + 1) * RPC],
                    hw2T_sb[:, c * RPC:(c + 1) * RPC],
                    AF.Tanh, bias=w_sum[:, c:c + 1],
                ))
            psS = psp.tile([128, 1], F32, tag="psS")
            for c in range(KC):
                nc.tensor.matmul(
                    psS[:, :], tT[:, c * RPC:(c + 1) * RPC], wa_sb[:, c:c + 1],
                    start=(c == 0), stop=(c == KC - 1),
                )
            if t >= 2 and not no_comm:
                lw2a = nc.scalar.wait_ge(lsem2, 64 * (t - 1))
                add_dep_helper(lw2a.ins, tanh_insts[-1].ins, False, "anchor lsem2 act")
            e_inst = nc.scalar.activation(pay2[par][:, 8:9], psS[:, :], AF.Exp)
            if t >= 2 and not no_comm:
                add_dep_helper(e_inst.ins, lw2a.ins, False, "pay2 WAR act")

            # ---- numer partial: H_own^T @ e (K = 128 rows) ----
            psN = psp.tile([128, KC], F32, tag="psN")
            if wdt != F32:
                e_w = sp.tile([RPC, 1], wdt, tag="e_w")
                nc.vector.tensor_copy(e_w[:, :], pay2[par][:, 8:9])
                e_rhs = e_w[:, :]
            else:
                e_rhs = pay2[par][:, 8:9]
            for m in range(KC):
                nc.tensor.matmul(
                    psN[:, m:m + 1], hrows_sb[:, m * 128:(m + 1) * 128], e_rhs,
                    start=True, stop=True,
                )
            if t >= 2 and not no_comm:
                lw2v = nc.vector.wait_ge(lsem2, 64 * (t - 1))
                add_dep_helper(lw2v.ins, i2.ins, False, "anchor lsem2 dve")
            cp2 = nc.vector.tensor_copy(pay2[par][:, 0:8], psN[:, :])
            if t >= 2 and not no_comm:
                add_dep_helper(cp2.ins, lw2v.ins, False, "pay2 WAR dve")

            # ---- fire SYNC2 ----
            if not no_comm:
                _trigger(nc, chain, [cp2.ins, e_inst.ins])
                w2_inst = nc.vector.wait_ge(sem2, 8 * (t + 1))
                add_dep_helper(w2_inst.ins, cp2.ins, False, "anchor sem2 wait")

            numer = sp.tile([128, KC], F32, tag="numer")
            r2 = rec2[par][:, :].rearrange("p (s c) -> p c s", s=QUAD)
            i3 = nc.vector.reduce_sum(numer[:, :], r2[:, 0:8, :], axis=mybir.AxisListType.X)
            zcol = sp.tile([128, 1], F32, tag="zcol")
            i4 = nc.vector.reduce_sum(zcol[:, :], r2[:, 8:9, :], axis=mybir.AxisListType.X)
            if not no_comm:
                add_dep_helper(i3.ins, w2_inst.ins, False, "gate recv2")
                add_dep_helper(i4.ins, w2_inst.ins, False, "gate recv2")

            psZ = psp.tile([128, 1], F32, tag="psZ")
            nc.tensor.matmul(psZ[:, :], ones_sb[:, :], zcol[:, :], start=True, stop=True)
            rz = sp.tile([128, 1], F32, tag="rz")
            nc.vector.reciprocal(rz[:, :], psZ[:, :])

            sub = sp.tile([128, KC], F32, tag="sub")
            nc.vector.tensor_scalar_mul(sub[:, :], numer[:, :], rz[:, :])
            hcol_new = sp.tile([128, KC], F32, tag="hcol")
            nc.vector.tensor_add(hcol_new[:, :], hnew_full[:, :], csum_sb[:, :])
            nc.vector.tensor_sub(hcol_new[:, :], hcol_new[:, :], sub[:, :])
            hcol = hcol_new

            nc.sync.dma_start(hout.ap()[t], hcol[:, :])

    nc.compile()
    return nc


def prep_in_maps(inputs: dict, T: int, wdt_np=ml_dtypes.bfloat16) -> list[dict]:
    X = np.asarray(inputs["inputs"], np.float32).reshape(T, IN_DIM)
    H = np.asarray(inputs["hiddn_state_mat"], np.float32)
    W_ioux = np.asarray(inputs["W_ioux"], np.float32)
    W_iouh = np.asarray(inputs["W_iouh"], np.float32)
    W_fx = np.asarray(inputs["W_fx"], np.float32)
    W_fh = np.asarray(inputs["W_fh"], np.float32)
    Wa = np.asarray(inputs["Wa"], np.float32).reshape(MEM)
    W_attnh = np.asarray(inputs["W_attnh"], np.float32)
    b_iou = (np.asarray(inputs["b_ioux"], np.float32)
             + np.asarray(inputs["b_iouh"], np.float32))
    b_f = (np.asarray(inputs["b_fx"], np.float32)
           + np.asarray(inputs["b_fh"], np.float32))
    b_attnh = np.asarray(inputs["b_attnh"], np.float32)

    W1 = W_attnh[:MEM]
    W2 = W_attnh[MEM:]

    xT_l = np.ascontiguousarray(
        X.T.reshape(KC, 128, T).transpose(1, 0, 2).reshape(128, KC * T))
    w2t = np.zeros((128, KC * KC * 128), np.float32)
    for c in range(KC):
        for k in range(KC):
            w2t[:, (c * KC + k) * 128:(c * KC + k + 1) * 128] = \
                W2[128 * k:128 * (k + 1), 128 * c:128 * (c + 1)]
    hT_l = np.ascontiguousarray(
        H.T.reshape(KC, 128, MROWS).transpose(1, 0, 2).reshape(128, KC * MROWS))
    wa_l = np.ascontiguousarray(Wa.reshape(KC, 128).T).astype(wdt_np)
    b2_l = np.ascontiguousarray(b_attnh.reshape(KC, 128).T)

    gate_w = [W_iouh[:, 0:MEM], W_iouh[:, MEM:2 * MEM], W_iouh[:, 2 * MEM:], W_fh]
    gate_wx = [W_ioux[:, 0:MEM], W_ioux[:, MEM:2 * MEM], W_ioux[:, 2 * MEM:], W_fx]
    gate_b = [b_iou[0:MEM], b_iou[MEM:2 * MEM], b_iou[2 * MEM:], b_f]

    maps = []
    for j in range(QUAD):
        wrec = np.zeros((128, 4 * KC * OC * 128), np.float32)
        wxl = np.zeros((128, 4 * KC * OC * 128), np.float32)
        for g in range(4):
            for k in range(KC):
                for o in range(OC):
                    idx = (g * KC + k) * OC + o
                    sl = np.s_[:, idx * 128:(idx + 1) * 128]
                    col0 = 256 * j + 128 * o
                    wrec[sl] = gate_w[g][128 * k:128 * (k + 1), col0:col0 + 128]
                    wxl[sl] = gate_wx[g][128 * k:128 * (k + 1), col0:col0 + 128]
        w1t = np.zeros((128, KC * OC * 128), np.float32)
        for m in range(KC):
            for o in range(OC):
                idx = m * OC + o
                row0 = 256 * j + 128 * o
                w1t[:, idx * 128:(idx + 1) * 128] = \
                    W1[row0:row0 + 128, 128 * m:128 * (m + 1)]
        bias_x = np.stack(
            [gate_b[g][256 * j + 128 * o:256 * j + 128 * (o + 1)]
             for g in range(4) for o in range(OC)], axis=1)
        mask = np.zeros((128, 16), np.float32)
        mask[:, 2 * j] = 1.0        # one-hot for chunk 2j in cols 0:8
        mask[:, 8 + 2 * j + 1] = 1.0  # one-hot for chunk 2j+1 in cols 8:16
        Hown = H[RPC * j:RPC * (j + 1)]  # [128, 1024]
        hTown_l = np.ascontiguousarray(
            Hown.T.reshape(KC, 128, RPC).transpose(1, 0, 2).reshape(128, KC * RPC))
        maps.append({
            "xT": xT_l, "wx": wxl,
            "wrec": wrec.astype(wdt_np), "w1t": w1t.astype(wdt_np),
            "w2t": w2t, "hT": hT_l, "hTown": hTown_l,
            "hrows": np.ascontiguousarray(Hown).astype(wdt_np),
            "wa": wa_l, "bias_x": np.ascontiguousarray(bias_x),
            "bias2": b2_l, "mask": mask,
        })
    return maps + maps  # quad replicated on cores 4-7


def postprocess(hout_core0: np.ndarray, T: int) -> np.ndarray:
    return np.ascontiguousarray(
        hout_core0.transpose(0, 2, 1).reshape(T, MEM)).astype(np.float32)


def kernel(**inputs) -> np.ndarray:
    from concourse.bass_utils import run_bass_kernel_spmd

    T = int(np.asarray(inputs["inputs"]).shape[0])
    nc = build_nc(T)
    in_maps = prep_in_maps(inputs, T)
    res = run_bass_kernel_spmd(nc, in_maps, core_ids=list(range(N_CORES)))
    hout = np.asarray(res.results[0]["hout"]).reshape(T, 128, KC)
    return postprocess(hout, T)


# revision 5
# speedup vs baseline: 1.8281x; 1.8281x over previous
"""ChildSum TreeLSTM + attention — quad-local SPMD Trainium2 kernel.

v2: all communication stays inside a 4-core quad (cross-die remote DMA
on this platform costs ~8x a same-die send and is flaky). Cores 0-3 and
4-7 run identical replicated quads; output is read from core 0.

Per-quad sharding (j = core % 4):
  - mem 1024 -> 4 slices of 256 (own chunks 2j, 2j+1 in [128, 8] col form)
  - attention rows 512 -> 128 rows per core
  - per step: ONE all-to-all [h one-hot | w partial] [128,16]; the
    whole attention epilogue (tanh/scores over ALL 512 rows, exp, Z,
    numer, h_att) is then computed locally on every core
  - SWDGE preps are hoisted before the payload writes (descriptors are
    address-only; data is read at trigger time), so only trigger+flight
    is on the critical path.
  - big stationaries (wrec/w1t/hrows/tT/wa) in bf16: PE time here is
    dominated by LD_WEIGHTS, ~1.5x cheaper in bf16.
"""
import numpy as np
from contextlib import ExitStack

import ml_dtypes

import concourse.bass as bass
import concourse.tile as tile
from concourse import bacc, mybir
from concourse.bass import create_sync_update
from concourse.tile_rust import add_dep_helper

F32 = mybir.dt.float32
BF16 = mybir.dt.bfloat16
AF = mybir.ActivationFunctionType
N_CORES = 8
QUAD = 4
MEM = 1024
IN_DIM = 1024
MROWS = 512
KC = MEM // 128          # 8 col chunks
OC = 2                   # own mem chunks per core (256 dims)
RPC = MROWS // QUAD      # 128 attention rows per core

_EXTERNAL_SEMS: list = []
_OrigCoreSim = tile.CoreSim


class _SchedCoreSim(_OrigCoreSim):
    def __init__(self, *a, **kw):
        super().__init__(*a, **kw)
        for sem in _EXTERNAL_SEMS:
            self.update_semaphore(create_sync_update(sem, 1 << 22))


tile.CoreSim = _SchedCoreSim


def _prep4(nc, out_slot_of, in_ap, remote_sem, local_sem, chain):
    """4 single-dest intra-quad broadcasts: slot k -> peer (own_tpb ^ k).

    Descriptors only; the caller fires them later with trigger_dma.
    Preps of consecutive syncs must hit the SWDGE ring in trace order.
    """
    prev = chain[0]
    for k in range(QUAD):
        rdests = [None] * 8
        rdests[k] = (0, k)
        inst = nc.gpsimd.remote_dma_broadcast(
            out_ap=out_slot_of(k),
            in_ap=in_ap,
            remote_sem=remote_sem,
            local_sem=local_sem,
            rdests=rdests,
        )
        if prev is not None:
            add_dep_helper(inst.ins, prev, False, "swdge ring order")
        prev = inst.ins
    chain[0] = prev
    return prev


def _trigger(nc, chain, payload_insts):
    trig = nc.gpsimd.trigger_dma(count=None)
    add_dep_helper(trig.ins, chain[0], False, "swdge ring order")
    for pi in payload_insts:
        # cross-engine (DVE/ACT -> Pool): needs a real semaphore edge
        add_dep_helper(trig.ins, pi, True, "payload ready")
    chain[0] = trig.ins
    return trig


def build_nc(T: int, wdt=BF16, t_run: int | None = None, no_comm: bool = False):
    del _EXTERNAL_SEMS[:]
    nc = bacc.Bacc()

    dp = lambda n, s, dt=F32: nc.declare_dram_parameter(n, s, dt, isOutput=False)
    xT = dp("xT", [128, KC * T])
    wx = dp("wx", [128, 4 * KC * OC * 128])
    wrec = dp("wrec", [128, 4 * KC * OC * 128], wdt)
    w1t = dp("w1t", [128, KC * OC * 128], wdt)
    w2t = dp("w2t", [128, KC * KC * 128])
    hTs = dp("hT", [128, KC * MROWS])
    hrowsF = dp("hrowsF", [128, QUAD * MEM], wdt)
    wa = dp("wa", [128, KC], wdt)
    bias_x = dp("bias_x", [128, 4 * OC])
    bias2 = dp("bias2", [128, KC])
    mask = dp("mask", [128, 16])  # cols 0:8 one-hot 2j, cols 8:16 one-hot 2j+1
    hout = nc.declare_dram_parameter("hout", [T, 128, KC], F32, isOutput=True)

    with tile.TileContext(nc) as tc, ExitStack() as ctx:
        sem1 = ctx.enter_context(nc.semaphore("rdma_sem1"))
        lsem1 = ctx.enter_context(nc.semaphore("rdma_lsem1"))
        _EXTERNAL_SEMS.extend([sem1, lsem1])

        comm = ctx.enter_context(tc.tile_pool(name="comm", bufs=1))
        pay1 = [comm.tile([128, 16], F32, name=f"pay1_{p}", tag=f"pay1_{p}") for p in range(2)]
        rec1 = [comm.tile([128, 16 * QUAD], F32, name=f"rec1_{p}", tag=f"rec1_{p}") for p in range(2)]

        const = ctx.enter_context(tc.tile_pool(name="const", bufs=1))
        wrec_sb = const.tile([128, 4 * KC * OC * 128], wdt, tag="wrec")
        w1t_sb = const.tile([128, KC * OC * 128], wdt, tag="w1t")
        hrows_sb = const.tile([128, QUAD * MEM], wdt, tag="hrowsF")
        wa_sb = const.tile([128, KC], wdt, tag="wa")
        hw2T_sb = const.tile([128, KC * MROWS], F32, tag="hw2T")
        xproj_sb = const.tile([128, 4 * OC * T], F32, tag="xproj")
        csum_sb = const.tile([128, KC], F32, tag="csum")
        ones_sb = const.tile([128, 128], F32, tag="ones")
        mask_sb = const.tile([128, 16], F32, tag="mask")

        nc.sync.dma_start(wrec_sb[:, :], wrec.ap())
        nc.sync.dma_start(w1t_sb[:, :], w1t.ap())
        nc.sync.dma_start(hrows_sb[:, :], hrowsF.ap())
        nc.sync.dma_start(wa_sb[:, :], wa.ap())
        nc.sync.dma_start(mask_sb[:, :], mask.ap())
        nc.vector.memset(ones_sb[:, :], 1.0)
        for p in range(2):
            nc.vector.memset(pay1[p][:, :], 0.0)
        if no_comm:
            for p in range(2):
                nc.vector.memset(rec1[p][:, :], 1.0)

        # ---------- device precompute ----------
        with tc.tile_pool(name="pre", bufs=1) as pre, \
             tc.tile_pool(name="prepsum", bufs=1, space="PSUM") as pps:
            xT_sb = pre.tile([128, KC * T], F32, tag="xT")
            wx_sb = pre.tile([128, 4 * KC * OC * 128], F32, tag="wx")
            w2t_sb = pre.tile([128, KC * KC * 128], F32, tag="w2t")
            hT_sb = pre.tile([128, KC * MROWS], F32, tag="hT")
            bx_sb = pre.tile([128, 4 * OC], F32, tag="bias_x")
            b2_sb = pre.tile([128, KC], F32, tag="bias2")
            nc.sync.dma_start(xT_sb[:, :], xT.ap())
            nc.sync.dma_start(wx_sb[:, :], wx.ap())
            nc.sync.dma_start(w2t_sb[:, :], w2t.ap())
            nc.sync.dma_start(hT_sb[:, :], hTs.ap())
            nc.sync.dma_start(bx_sb[:, :], bias_x.ap())
            nc.sync.dma_start(b2_sb[:, :], bias2.ap())

            # xproj[(g,o)]: [128, T] = sum_k Wx tile (g,k,o)^T @ xT[k]
            for g in range(4):
                for o in range(OC):
                    ps = pps.tile([128, T], F32, tag="ps_x")
                    for k in range(KC):
                        idx = (g * KC + k) * OC + o
                        nc.tensor.matmul(
                            ps[:, :],
                            wx_sb[:, idx * 128:(idx + 1) * 128],
                            xT_sb[:, k * T:(k + 1) * T],
                            start=(k == 0), stop=(k == KC - 1),
                        )
                        nc.vector.tensor_scalar_add(
                            xproj_sb[:, (g * OC + o) * T:(g * OC + o + 1) * T],
                            ps[:, :], bx_sb[:, g * OC + o:g * OC + o + 1],
                        ) if k == KC - 1 else None

            # hw2T c-chunk: [128, MROWS] = sum_k W2[c,k]^T @ hT[k] (all rows)
            for c in range(KC):
                ps2 = pps.tile([128, MROWS], F32, tag="ps_h")
                for k in range(KC):
                    nc.tensor.matmul(
                        ps2[:, :],
                        w2t_sb[:, (c * KC + k) * 128:(c * KC + k + 1) * 128],
                        hT_sb[:, k * MROWS:(k + 1) * MROWS],
                        start=(k == 0), stop=(k == KC - 1),
                    )
                nc.vector.tensor_scalar_add(
                    hw2T_sb[:, c * MROWS:(c + 1) * MROWS], ps2[:, :], b2_sb[:, c:c + 1]
                )

            for m in range(KC):
                nc.vector.reduce_sum(
                    csum_sb[:, m:m + 1],
                    hT_sb[:, m * MROWS:(m + 1) * MROWS],
                    axis=mybir.AxisListType.X,
                )

        # ---------- state & per-step pools ----------
        sp = ctx.enter_context(tc.tile_pool(name="step", bufs=2))
        psp = ctx.enter_context(tc.tile_pool(name="spsum", bufs=1, space="PSUM"))

        chain = [None]
        hcol = sp.tile([128, KC], F32, tag="hcol")
        ccol = sp.tile([128, OC], F32, tag="ccol")
        nc.vector.memset(hcol[:, :], 0.0)
        nc.vector.memset(ccol[:, :], 0.0)

        for t in range(t_run if t_run is not None else T):
            par = t & 1
            # hoisted SYNC1 preps (descriptors only; data read at trigger)
            if not no_comm:
                _prep4(nc, lambda k: rec1[par][:, k * 16:(k + 1) * 16],
                       pay1[par][:, 0:16], sem1, lsem1, chain)

            # ---- phase A: gate pre-activations for own 2 chunks ----
            if wdt != F32:
                hcol_w = sp.tile([128, KC], wdt, tag="hcol_w")
                nc.vector.tensor_copy(hcol_w[:, :], hcol[:, :])
            else:
                hcol_w = hcol
            psA = psp.tile([128, 4 * OC], F32, tag="psA")
            for g in range(4):
                for o in range(OC):
                    for k in range(KC):
                        idx = (g * KC + k) * OC + o
                        nc.tensor.matmul(
                            psA[:, g * OC + o:g * OC + o + 1],
                            wrec_sb[:, idx * 128:(idx + 1) * 128],
                            hcol_w[:, k:k + 1],
                            start=(k == 0), stop=(k == KC - 1),
                        )
            # ---- gates ----
            gates = sp.tile([128, 4 * OC], F32, tag="gates")
            for g, fn in ((0, AF.Sigmoid), (1, AF.Sigmoid), (2, AF.Tanh), (3, AF.Sigmoid)):
                for o in range(OC):
                    col = g * OC + o
                    nc.scalar.activation(
                        gates[:, col:col + 1], psA[:, col:col + 1], fn,
                        bias=xproj_sb[:, col * T + t:col * T + t + 1],
                    )
            iu = sp.tile([128, OC], F32, tag="iu")
            nc.vector.tensor_mul(iu[:, :], gates[:, 0:2], gates[:, 4:6])
            ccol_new = sp.tile([128, OC], F32, tag="ccol")
            nc.vector.tensor_mul(ccol_new[:, :], gates[:, 6:8], ccol[:, :])
            nc.vector.tensor_add(ccol_new[:, :], ccol_new[:, :], iu[:, :])
            ccol = ccol_new
            tanh_c = sp.tile([128, OC], F32, tag="tanh_c")
            nc.scalar.activation(tanh_c[:, :], ccol[:, :], AF.Tanh)
            h_new = sp.tile([128, OC], F32, tag="h_new")
            hn_inst = nc.vector.tensor_mul(h_new[:, :], gates[:, 2:4], tanh_c[:, :])

            # ---- w-gemv: psW[:, m] = sum_o W1[own o]^T @ h_new[:, o] ----
            psW = psp.tile([128, KC], F32, tag="psW")
            if wdt != F32:
                h_new_w = sp.tile([128, OC], wdt, tag="h_new_w")
                nc.vector.tensor_copy(h_new_w[:, :], h_new[:, :])
            else:
                h_new_w = h_new
            for m in range(KC):
                for o in range(OC):
                    idx = m * OC + o
                    nc.tensor.matmul(
                        psW[:, m:m + 1],
                        w1t_sb[:, idx * 128:(idx + 1) * 128],
                        h_new_w[:, o:o + 1],
                        start=(o == 0), stop=(o == OC - 1),
                    )
            if t >= 2 and not no_comm:
                lw1 = nc.vector.wait_ge(lsem1, 64 * (t - 1))
                add_dep_helper(lw1.ins, hn_inst.ins, False, "anchor lsem1 wait")

            tmpE = sp.tile([128, KC], F32, tag="tmpE")
            tmpO = sp.tile([128, KC], F32, tag="tmpO")
            mE = nc.vector.tensor_scalar_mul(tmpE[:, :], mask_sb[:, 0:8], h_new[:, 0:1])
            mO = nc.vector.tensor_scalar_mul(tmpO[:, :], mask_sb[:, 8:16], h_new[:, 1:2])
            cph = nc.vector.tensor_add(pay1[par][:, 0:8], tmpE[:, :], tmpO[:, :])
            cp1 = nc.vector.tensor_copy(pay1[par][:, 8:16], psW[:, :])
            if t >= 2 and not no_comm:
                add_dep_helper(cph.ins, lw1.ins, False, "pay1 WAR")
                add_dep_helper(cp1.ins, lw1.ins, False, "pay1 WAR")

            # ---- fire SYNC1 ----
            # cph precedes cp1 on the in-order DVE, so cp1 alone gates
            if not no_comm:
                _trigger(nc, chain, [cp1.ins])
                w1_inst = nc.vector.wait_ge(sem1, 8 * (t + 1))
                add_dep_helper(w1_inst.ins, cp1.ins, False, "anchor sem1 wait")

            hnew_full = sp.tile([128, KC], F32, tag="hnew_full")
            r1 = rec1[par][:, :].rearrange("p (s c) -> p c s", s=QUAD)
            i1 = nc.vector.reduce_sum(hnew_full[:, :], r1[:, 0:8, :], axis=mybir.AxisListType.X)
            w_sum = sp.tile([128, KC], F32, tag="w_sum")
            i2 = nc.vector.reduce_sum(w_sum[:, :], r1[:, 8:16, :], axis=mybir.AxisListType.X)
            if not no_comm:
                add_dep_helper(i1.ins, w1_inst.ins, False, "gate recv1")
                add_dep_helper(i2.ins, w1_inst.ins, False, "gate recv1")

            # ---- attention, ALL 512 rows local: tT = tanh(hw2T + w) ----
            # scores MMs interleaved after each chunk's tanh (PE runs under ACT)
            tT = sp.tile([128, KC * MROWS], wdt, tag="tT")
            psS = psp.tile([128, QUAD], F32, tag="psS")
            for m in range(KC):
                nc.scalar.activation(
                    tT[:, m * MROWS:(m + 1) * MROWS],
                    hw2T_sb[:, m * MROWS:(m + 1) * MROWS],
                    AF.Tanh, bias=w_sum[:, m:m + 1],
                )
                for c in range(QUAD):
                    nc.tensor.matmul(
                        psS[:, c:c + 1],
                        tT[:, m * MROWS + c * 128:m * MROWS + (c + 1) * 128],
                        wa_sb[:, m:m + 1],
                        start=(m == 0), stop=(m == KC - 1),
                    )
            e4 = sp.tile([128, QUAD], F32, tag="e4")
            nc.scalar.activation(e4[:, :], psS[:, :], AF.Exp)

            # ---- Z and numer, local ----
            zcol = sp.tile([128, 1], F32, tag="zcol")
            nc.vector.reduce_sum(zcol[:, :], e4[:, :], axis=mybir.AxisListType.X)
            psZ = psp.tile([128, 1], F32, tag="psZ")
            nc.tensor.matmul(psZ[:, :], ones_sb[:, :], zcol[:, :], start=True, stop=True)
            rz = sp.tile([128, 1], F32, tag="rz")
            nc.vector.reciprocal(rz[:, :], psZ[:, :])

            if wdt != F32:
                e_w = sp.tile([128, QUAD], wdt, tag="e_w")
                nc.vector.tensor_copy(e_w[:, :], e4[:, :])
            else:
                e_w = e4
            psN = psp.tile([128, KC], F32, tag="psN")
            for m in range(KC):
                for c in range(QUAD):
                    nc.tensor.matmul(
                        psN[:, m:m + 1],
                        hrows_sb[:, c * MEM + m * 128:c * MEM + (m + 1) * 128],
                        e_w[:, c:c + 1],
                        start=(c == 0), stop=(c == QUAD - 1),
                    )

            sub = sp.tile([128, KC], F32, tag="sub")
            nc.vector.tensor_scalar_mul(sub[:, :], psN[:, :], rz[:, :])
            hcol_new = sp.tile([128, KC], F32, tag="hcol")
            nc.vector.tensor_add(hcol_new[:, :], hnew_full[:, :], csum_sb[:, :])
            nc.vector.tensor_sub(hcol_new[:, :], hcol_new[:, :], sub[:, :])
            hcol = hcol_new

            nc.sync.dma_start(hout.ap()[t], hcol[:, :])

    nc.compile()
    return nc


def prep_in_maps(inputs: dict, T: int, wdt_np=ml_dtypes.bfloat16) -> list[dict]:
    X = np.asarray(inputs["inputs"], np.float32).reshape(T, IN_DIM)
    H = np.asarray(inputs["hiddn_state_mat"], np.float32)
    W_ioux = np.asarray(inputs["W_ioux"], np.float32)
    W_iouh = np.asarray(inputs["W_iouh"], np.float32)
    W_fx = np.asarray(inputs["W_fx"], np.float32)
    W_fh = np.asarray(inputs["W_fh"], np.float32)
    Wa = np.asarray(inputs["Wa"], np.float32).reshape(MEM)
    W_attnh = np.asarray(inputs["W_attnh"], np.float32)
    b_iou = (np.asarray(inputs["b_ioux"], np.float32)
             + np.asarray(inputs["b_iouh"], np.float32))
    b_f = (np.asarray(inputs["b_fx"], np.float32)
           + np.asarray(inputs["b_fh"], np.float32))
    b_attnh = np.asarray(inputs["b_attnh"], np.float32)

    W1 = W_attnh[:MEM]
    W2 = W_attnh[MEM:]

    xT_l = np.ascontiguousarray(
        X.T.reshape(KC, 128, T).transpose(1, 0, 2).reshape(128, KC * T))
    w2t = np.zeros((128, KC * KC * 128), np.float32)
    for c in range(KC):
        for k in range(KC):
            w2t[:, (c * KC + k) * 128:(c * KC + k + 1) * 128] = \
                W2[128 * k:128 * (k + 1), 128 * c:128 * (c + 1)]
    hT_l = np.ascontiguousarray(
        H.T.reshape(KC, 128, MROWS).transpose(1, 0, 2).reshape(128, KC * MROWS))
    wa_l = np.ascontiguousarray(Wa.reshape(KC, 128).T).astype(wdt_np)
    b2_l = np.ascontiguousarray(b_attnh.reshape(KC, 128).T)

    gate_w = [W_iouh[:, 0:MEM], W_iouh[:, MEM:2 * MEM], W_iouh[:, 2 * MEM:], W_fh]
    gate_wx = [W_ioux[:, 0:MEM], W_ioux[:, MEM:2 * MEM], W_ioux[:, 2 * MEM:], W_fx]
    gate_b = [b_iou[0:MEM], b_iou[MEM:2 * MEM], b_iou[2 * MEM:], b_f]

    maps = []
    for j in range(QUAD):
        wrec = np.zeros((128, 4 * KC * OC * 128), np.float32)
        wxl = np.zeros((128, 4 * KC * OC * 128), np.float32)
        for g in range(4):
            for k in range(KC):
                for o in range(OC):
                    idx = (g * KC + k) * OC + o
                    sl = np.s_[:, idx * 128:(idx + 1) * 128]
                    col0 = 256 * j + 128 * o
                    wrec[sl] = gate_w[g][128 * k:128 * (k + 1), col0:col0 + 128]
                    wxl[sl] = gate_wx[g][128 * k:128 * (k + 1), col0:col0 + 128]
        w1t = np.zeros((128, KC * OC * 128), np.float32)
        for m in range(KC):
            for o in range(OC):
                idx = m * OC + o
                row0 = 256 * j + 128 * o
                w1t[:, idx * 128:(idx + 1) * 128] = \
                    W1[row0:row0 + 128, 128 * m:128 * (m + 1)]
        bias_x = np.stack(
            [gate_b[g][256 * j + 128 * o:256 * j + 128 * (o + 1)]
             for g in range(4) for o in range(OC)], axis=1)
        mask = np.zeros((128, 16), np.float32)
        mask[:, 2 * j] = 1.0        # one-hot for chunk 2j in cols 0:8
        mask[:, 8 + 2 * j + 1] = 1.0  # one-hot for chunk 2j+1 in cols 8:16
        hrowsF_l = np.ascontiguousarray(
            H.reshape(QUAD, 128, MEM).transpose(1, 0, 2).reshape(128, QUAD * MEM))
        maps.append({
            "xT": xT_l, "wx": wxl,
            "wrec": wrec.astype(wdt_np), "w1t": w1t.astype(wdt_np),
            "w2t": w2t, "hT": hT_l,
            "hrowsF": hrowsF_l.astype(wdt_np),
            "wa": wa_l, "bias_x": np.ascontiguousarray(bias_x),
            "bias2": b2_l, "mask": mask,
        })
    return maps + maps  # quad replicated on cores 4-7


def postprocess(hout_core0: np.ndarray, T: int) -> np.ndarray:
    return np.ascontiguousarray(
        hout_core0.transpose(0, 2, 1).reshape(T, MEM)).astype(np.float32)


def kernel(**inputs) -> np.ndarray:
    from concourse.bass_utils import run_bass_kernel_spmd

    T = int(np.asarray(inputs["inputs"]).shape[0])
    nc = build_nc(T)
    in_maps = prep_in_maps(inputs, T)
    res = run_bass_kernel_spmd(nc, in_maps, core_ids=list(range(N_CORES)))
    hout = np.asarray(res.results[0]["hout"]).reshape(T, 128, KC)
    return postprocess(hout, T)
